# revision 11
# baseline (speedup 1.0000x reference)
"""Rotary multi-head attention (b=8, n=1024, dim=768, heads=12, d_head=64)
on 8 Trainium2 NeuronCores, data-parallel over batch (1 batch row per core).

v3: fp16 operands on the scores path (10-bit mantissa ~ tf32 accuracy, but
2-byte so every 128-col stationary gets the fast-weight-load path), bf16 for
exp outputs / V (exp can overflow fp16 range). Host-side prep: X shipped
pre-transposed, rotary sin/cos tables precomputed, weights pre-cast — the
device does no transposes and no weight staging. Attention runs per head
with a 3-tag PSUM budget (scores x2 | AV accumulator | matmul accumulator)
and the AV accumulator is freed early via a PSUM->SBUF copy so heads
pipeline; QK production for pair t+1 fills PE gaps under pair t's softmax.
"""
import sys
import numpy as np

if '/opt/trn_rl_repo' not in sys.path:
    sys.path.insert(0, '/opt/trn_rl_repo')

B, N, DIM = 8, 1024, 768
HEADS, DHEAD = 12, 64
INNER = HEADS * DHEAD           # 768
SCALE = DHEAD ** -0.5           # 0.125
NCH = N // 128                  # 8 n-chunks
KCH = DIM // 128                # 6 contraction chunks
TCH = HEADS // 2                # 6 head pairs

_CACHE = {}


def _build():
    import concourse.mybir as mybir
    from concourse import bacc
    from concourse.tile import TileContext

    F32 = mybir.dt.float32
    F16 = mybir.dt.float16
    BF16 = mybir.dt.bfloat16
    AF = mybir.ActivationFunctionType

    nc = bacc.Bacc("TRN2", target_bir_lowering=False, debug=False, num_devices=8)

    xt_d = nc.dram_tensor("xt", [DIM, N], F16, kind="ExternalInput")
    wqkv_d = nc.dram_tensor("wqkv", [DIM, 3 * INNER], F16, kind="ExternalInput")
    wout_d = nc.dram_tensor("wout", [INNER, DIM], F16, kind="ExternalInput")
    bout_d = nc.dram_tensor("bout", [DIM], F32, kind="ExternalInput")
    sin_d = nc.dram_tensor("sintab", [128, N], F16, kind="ExternalInput")
    cos_d = nc.dram_tensor("costab", [128, N], F16, kind="ExternalInput")
    y_d = nc.dram_tensor("y", [N, DIM], F32, kind="ExternalOutput")
    den_d = nc.dram_tensor("den_scr", [HEADS, N], F32)
    rcp_d = nc.dram_tensor("rcp_scr", [HEADS, N], F32)

    with TileContext(nc) as tc:
        with tc.tile_pool(name="wp", bufs=1) as wp, \
             tc.tile_pool(name="big", bufs=1) as big, \
             tc.tile_pool(name="tp", bufs=2) as tp, \
             tc.tile_pool(name="epool", bufs=1) as epool, \
             tc.tile_pool(name="misc", bufs=1) as misc, \
             tc.tile_pool(name="ps", bufs=1, space="PSUM") as ps:

            def psum(name, tag, bufs):
                return ps.tile([128, N], F32, name=name, tag=tag, bufs=bufs)

            # ---- input DMAs: interleave xt/wqkv pairs on opposite queues so
            # the k=0 chunks (needed first) land first on both queues.
            q_of = [nc.sync, nc.scalar]
            xt = [wp.tile([128, N], F16, name=f"xt{k}", tag=f"xt{k}")
                  for k in range(KCH)]
            wqkv_sb = [wp.tile([128, 3 * INNER], F16, name=f"wqkv_{k}",
                               tag=f"wqkv_{k}") for k in range(KCH)]
            for k in range(KCH):
                q_of[k % 2].dma_start(xt[k][:], xt_d[k * 128:(k + 1) * 128, :])
                q_of[(k + 1) % 2].dma_start(wqkv_sb[k][:],
                                            wqkv_d[k * 128:(k + 1) * 128, :])
            sin_sb = misc.tile([128, N], F16, name="sin_sb", tag="sin_sb")
            nc.sync.dma_start(sin_sb[:], sin_d.ap())
            cos_sb = misc.tile([128, N], F16, name="cos_sb", tag="cos_sb")
            nc.scalar.dma_start(cos_sb[:], cos_d.ap())

            wout_sb = [wp.tile([128, DIM], F16, name=f"wout_{k}",
                               tag=f"wout_{k}") for k in range(KCH)]
            for k in range(KCH):
                q_of[k % 2].dma_start(wout_sb[k][:],
                                      wout_d[k * 128:(k + 1) * 128, :])
            b_bcast = misc.tile([128, DIM], F32, name="b_bcast", tag="b_bcast")
            nc.scalar.dma_start(b_bcast[:],
                                bout_d.ap().unsqueeze(0).broadcast_to((128, DIM)))

            # ---- vaug: per n-chunk, [128, h*128 + (64 V | 1 ones | 63 zero)]
            vaug = [big.tile([128, HEADS * 128], BF16, name=f"vaug{i}",
                             tag=f"vaug{i}") for i in range(NCH)]
            for i in range(NCH):
                nc.gpsimd.memset(vaug[i][:], 0.0)
                for h in range(HEADS):
                    nc.gpsimd.memset(vaug[i][:, h * 128 + 64:h * 128 + 65], 1.0)

            # ---- V phase: natural layout, stationary = xt chunk (FWL);
            # ping-pong between the acc and (not-yet-used) avs PSUM tags so
            # chunk i+1's matmuls overlap chunk i's PSUM->SBUF copies.
            for i in range(NCH):
                acc = psum(f"vp_{i}", "acc" if i % 2 == 0 else "avs", 1)
                for k in range(KCH):
                    lhs = xt[k][:, i * 128:(i + 1) * 128]
                    for half in range(2):
                        nc.tensor.matmul(
                            acc[:, half * 512:half * 512 + 384],
                            lhs,
                            wqkv_sb[k][:, 2 * INNER + half * 384:
                                       2 * INNER + (half + 1) * 384],
                            start=(k == 0), stop=(k == KCH - 1))
                for half in range(2):
                    dst = vaug[i].rearrange("p (h c) -> p h c", c=128)[
                        :, 6 * half:6 * (half + 1), 0:64]
                    nc.vector.tensor_copy(
                        dst, acc[:, half * 512:half * 512 + 384]
                        .rearrange("p (h d) -> p h d", d=64))

            # ---- interleaved: per pair t produce QK chunks (c=t, 6+t) with
            # rotary, then attention for pair t; the scheduler overlaps pair
            # t's attention with pair t+1's QK production.
            ao = [big.tile([128, N], F16, name=f"ao{t}", tag=f"ao{t}")
                  for t in range(TCH)]
            qkl = {}

            def emit_qk_pair(tq):
                for c in (tq, 6 + tq):
                    qkc = big.tile([128, N], F16, name=f"qk{c}",
                                   tag="qkA" if c < 6 else "qkB", bufs=2)
                    qkl[(tq, c >= 6)] = qkc
                    acc = psum(f"qkp_{c}", "acc", 1)
                    for k in range(KCH):
                        lhs = wqkv_sb[k][:, c * 128:(c + 1) * 128]
                        for half in range(2):
                            sl = slice(half * 512, (half + 1) * 512)
                            nc.tensor.matmul(acc[:, sl], lhs, xt[k][:, sl],
                                             start=(k == 0), stop=(k == KCH - 1))
                    qraw = tp.tile([128, N], F16, name=f"qraw_{c}",
                                   tag="qraw", bufs=2)
                    nc.vector.tensor_copy(qraw[:], acc[:])
                    # rotary: qkc <- qraw*cos + swap(qraw)*sin_signed, where
                    # the pair swap is two partition-strided SBUF DMAs and
                    # the per-row sign of rotate_every_two is folded into
                    # the host-built sin table.
                    rps = tp.tile([128, N], F16, name=f"rps_{c}", tag="rps",
                                  bufs=2)
                    qv = qraw.rearrange("(p s) n -> p s n", s=2)
                    rv = rps.rearrange("(p s) n -> p s n", s=2)
                    nc.scalar.dma_start(rv[:, 0, :], qv[:, 1, :])
                    nc.scalar.dma_start(rv[:, 1, :], qv[:, 0, :])
                    t1 = tp.tile([128, N], F16, name=f"t1_{c}", tag="t1",
                                 bufs=2)
                    nc.vector.tensor_mul(t1[:], qraw[:], cos_sb[:])
                    nc.vector.tensor_mul(qkc[:], rps[:], sin_sb[:])
                    nc.vector.tensor_add(qkc[:], qkc[:], t1[:])

            def emit_attention(t):
                # attention for pair t, one head at a time (one behind QK
                # production). avs is freed early via the av_sb copy so the
                # next head's accumulation overlaps this head's normalize.
                qkQ = qkl[(t, False)]
                qkK = qkl[(t, True)]
                for half in range(2):
                    h = 2 * t + half
                    hs = slice(half * 64, (half + 1) * 64)
                    avs = psum(f"av_{h}", "avs", 1)
                    for jc in range(NCH):
                        kt_slice = qkK[hs, jc * 128:(jc + 1) * 128]
                        e = epool.tile([128, N], BF16, name=f"e_{h}_{jc}",
                                       tag="e", bufs=4)
                        stp = psum(f"st_{h}_{jc}", "stp", 2)
                        for hf in range(2):
                            sl = slice(hf * 512, (hf + 1) * 512)
                            nc.tensor.matmul(stp[:, sl], kt_slice,
                                             qkQ[hs, sl],
                                             start=True, stop=True)
                        nc.scalar.activation(e[:], stp[:], AF.Exp, scale=SCALE)
                        v_sl = vaug[jc][:, h * 128:(h + 1) * 128]
                        for hf in range(2):
                            sl = slice(hf * 512, (hf + 1) * 512)
                            nc.tensor.matmul(avs[:, sl], v_sl, e[:, sl],
                                             start=(jc == 0),
                                             stop=(jc == NCH - 1))
                    # early evacuation: rows 0:64 = out^T, row 64 = denom
                    av_sb = tp.tile([65, N], F32, name=f"avsb_{h}",
                                    tag="avsb", bufs=2)
                    nc.vector.tensor_copy(av_sb[:], avs[0:65, :])
                    # normalize: den -> dram -> [128,8] recip -> dram -> row
                    # -> gpsimd bcast [64,N]; multiply into ao (fp16).
                    nc.sync.dma_start(den_d.ap()[h].unsqueeze(0),
                                      av_sb[64:65, :])
                    dsq = tp.tile([128, 8], F32, name=f"dsq_{h}", tag="dsq",
                                  bufs=2)
                    nc.sync.dma_start(
                        dsq[:], den_d.ap()[h].rearrange("(p f) -> p f", f=8))
                    nc.vector.reciprocal(dsq[:], dsq[:])
                    nc.sync.dma_start(
                        rcp_d.ap()[h].rearrange("(p f) -> p f", f=8), dsq[:])
                    rep = tp.tile([64, N], F32, name=f"rep_{h}", tag="rep",
                                  bufs=2)
                    nc.sync.dma_start(
                        rep[:], rcp_d.ap()[h].unsqueeze(0).broadcast_to((64, N)))
                    nc.vector.tensor_mul(ao[t][hs, :], av_sb[0:64, :], rep[:])

            # ---- incremental output projection: as soon as pair t's ao is
            # normalized, project it (stationary = ao chunk, FWL) and fold
            # into an SBUF fp32 accumulator via DVE; only the last pair's
            # contribution sits in the kernel tail.
            y_acc = [big.tile([128, DIM], F32, name=f"yacc{i}", tag=f"yacc{i}")
                     for i in range(NCH)]

            def emit_outproj_pair(t):
                for i in range(NCH):
                    op = psum(f"op_{t}_{i}", "acc", 1)
                    lhs = ao[t][:, i * 128:(i + 1) * 128]
                    nc.tensor.matmul(op[:, 0:512], lhs, wout_sb[t][:, 0:512],
                                     start=True, stop=True)
                    nc.tensor.matmul(op[:, 512:768], lhs, wout_sb[t][:, 512:768],
                                     start=True, stop=True)
                    if t == 0:
                        nc.vector.tensor_add(y_acc[i][:], op[:, 0:768],
                                             b_bcast[:])
                    else:
                        nc.vector.tensor_add(y_acc[i][:], y_acc[i][:],
                                             op[:, 0:768])
                    if t == TCH - 1:
                        nc.sync.dma_start(y_d[i * 128:(i + 1) * 128, :],
                                          y_acc[i][:])

            emit_qk_pair(0)
            for t in range(1, TCH):
                emit_qk_pair(t)
                emit_attention(t - 1)
                emit_outproj_pair(t - 1)
            emit_attention(TCH - 1)
            emit_outproj_pair(TCH - 1)

    nc.compile()
    return nc


def get_nc():
    if 'nc' not in _CACHE:
        _CACHE['nc'] = _build()
    return _CACHE['nc']


def make_in_maps(inputs):
    F16 = np.float16
    x = np.asarray(inputs["x"], dtype=np.float32)
    pos = np.asarray(inputs["pos_emb"], dtype=np.float32).reshape(N, DHEAD)
    wqkv = np.ascontiguousarray(
        np.asarray(inputs["W_qkv"], dtype=np.float32).astype(F16))
    wout = np.ascontiguousarray(
        np.asarray(inputs["W_out"], dtype=np.float32).astype(F16))
    bout = np.ascontiguousarray(np.asarray(inputs["b_out"], dtype=np.float32))
    # rotary tables in the transposed [d=128, n] layout used on-device:
    # row m of a head-half uses sin(pos[n, (m%64)//2]), cos(pos[n, 32+(m%64)//2]).
    # rotate_every_two's sign pattern (-odd, +even source) is folded into the
    # sin table since the device does an unsigned pair-swap copy.
    d = np.arange(128) % 64
    sgn = np.where(np.arange(128) % 2 == 0, -1.0, 1.0).astype(np.float32)
    sintab = np.ascontiguousarray((sgn[:, None] * pos[:, d // 2].T).astype(F16))
    costab = np.ascontiguousarray(pos[:, 32 + d // 2].T.astype(F16))
    return [{"xt": np.ascontiguousarray(x[i].T.astype(F16)),
             "wqkv": wqkv, "wout": wout, "bout": bout,
             "sintab": sintab, "costab": costab} for i in range(B)]


def run(inputs, trace=False, **kwargs):
    """inputs: dict with full-shape arrays as in reference.setup_inputs()."""
    from concourse.bass_utils import run_bass_kernel_spmd
    nc = get_nc()
    res = run_bass_kernel_spmd(nc, make_in_maps(inputs),
                               core_ids=list(range(B)), trace=trace, **kwargs)
    out = np.stack([res.results[i]["y"] for i in range(B)], axis=0)
    return out, res


def kernel(**inputs):
    out, _ = run(inputs, trace=False)
    return out


# revision 13
# speedup vs baseline: 1.6812x; 1.6812x over previous
"""Rotary multi-head attention (b=8, n=1024, dim=768, heads=12, d_head=64)
on 8 Trainium2 NeuronCores, data-parallel over batch (1 batch row per core).

v3: fp16 operands on the scores path (10-bit mantissa ~ tf32 accuracy, but
2-byte so every 128-col stationary gets the fast-weight-load path), bf16 for
exp outputs / V (exp can overflow fp16 range). Host-side prep: X shipped
pre-transposed, rotary sin/cos tables precomputed, weights pre-cast — the
device does no transposes and no weight staging. Attention runs per head
with a 3-tag PSUM budget (scores x2 | AV accumulator | matmul accumulator)
and the AV accumulator is freed early via a PSUM->SBUF copy so heads
pipeline; QK production for pair t+1 fills PE gaps under pair t's softmax.
"""
import sys
import numpy as np

if '/opt/trn_rl_repo' not in sys.path:
    sys.path.insert(0, '/opt/trn_rl_repo')

B, N, DIM = 8, 1024, 768
HEADS, DHEAD = 12, 64
INNER = HEADS * DHEAD           # 768
SCALE = DHEAD ** -0.5           # 0.125
NCH = N // 128                  # 8 n-chunks
KCH = DIM // 128                # 6 contraction chunks
TCH = HEADS // 2                # 6 head pairs

_CACHE = {}


def _build():
    import concourse.mybir as mybir
    from concourse import bacc
    from concourse.tile import TileContext

    F32 = mybir.dt.float32
    F16 = mybir.dt.float16
    BF16 = mybir.dt.bfloat16
    AF = mybir.ActivationFunctionType

    nc = bacc.Bacc("TRN2", target_bir_lowering=False, debug=False, num_devices=8)

    xt_d = nc.dram_tensor("xt", [DIM, N], F16, kind="ExternalInput")
    wqkv_d = nc.dram_tensor("wqkv", [DIM, 3 * INNER], F16, kind="ExternalInput")
    wout_d = nc.dram_tensor("wout", [INNER, DIM], F16, kind="ExternalInput")
    bout_d = nc.dram_tensor("bout", [DIM], F32, kind="ExternalInput")
    sin_d = nc.dram_tensor("sintab", [128, N], F16, kind="ExternalInput")
    cos_d = nc.dram_tensor("costab", [128, N], F16, kind="ExternalInput")
    y_d = nc.dram_tensor("y", [N, DIM], F32, kind="ExternalOutput")
    den_d = nc.dram_tensor("den_scr", [HEADS, N], F32)
    rcp_d = nc.dram_tensor("rcp_scr", [HEADS, N], F32)

    with TileContext(nc) as tc:
        with tc.tile_pool(name="wp", bufs=1) as wp, \
             tc.tile_pool(name="big", bufs=1) as big, \
             tc.tile_pool(name="tp", bufs=2) as tp, \
             tc.tile_pool(name="epool", bufs=1) as epool, \
             tc.tile_pool(name="misc", bufs=1) as misc, \
             tc.tile_pool(name="ps", bufs=1, space="PSUM") as ps:

            def psum(name, tag, bufs):
                return ps.tile([128, N], F32, name=name, tag=tag, bufs=bufs)

            # ---- input DMAs: interleave xt/wqkv pairs on opposite queues so
            # the k=0 chunks (needed first) land first on both queues.
            q_of = [nc.sync, nc.scalar]
            xt = [wp.tile([128, N], F16, name=f"xt{k}", tag=f"xt{k}")
                  for k in range(KCH)]
            wqkv_sb = [wp.tile([128, 3 * INNER], F16, name=f"wqkv_{k}",
                               tag=f"wqkv_{k}") for k in range(KCH)]
            for k in range(KCH):
                q_of[k % 2].dma_start(xt[k][:], xt_d[k * 128:(k + 1) * 128, :])
                q_of[(k + 1) % 2].dma_start(wqkv_sb[k][:],
                                            wqkv_d[k * 128:(k + 1) * 128, :])
            sin_sb = misc.tile([128, N], F16, name="sin_sb", tag="sin_sb")
            nc.sync.dma_start(sin_sb[:], sin_d.ap())
            cos_sb = misc.tile([128, N], F16, name="cos_sb", tag="cos_sb")
            nc.scalar.dma_start(cos_sb[:], cos_d.ap())

            wout_sb = [wp.tile([128, DIM], F16, name=f"wout_{k}",
                               tag=f"wout_{k}") for k in range(KCH)]
            for k in range(KCH):
                q_of[k % 2].dma_start(wout_sb[k][:],
                                      wout_d[k * 128:(k + 1) * 128, :])
            b_bcast = misc.tile([128, DIM], F32, name="b_bcast", tag="b_bcast")
            nc.scalar.dma_start(b_bcast[:],
                                bout_d.ap().unsqueeze(0).broadcast_to((128, DIM)))

            # ---- vaug: per n-chunk, [128, h*128 + (64 V | 1 ones | 63 zero)]
            vaug = [big.tile([128, HEADS * 128], BF16, name=f"vaug{i}",
                             tag=f"vaug{i}") for i in range(NCH)]
            for i in range(NCH):
                nc.gpsimd.memset(vaug[i][:], 0.0)
                for h in range(HEADS):
                    nc.gpsimd.memset(vaug[i][:, h * 128 + 64:h * 128 + 65], 1.0)

            # ---- V phase: natural layout, stationary = xt chunk (FWL);
            # ping-pong between the acc and (not-yet-used) avs PSUM tags so
            # chunk i+1's matmuls overlap chunk i's PSUM->SBUF copies.
            for i in range(NCH):
                acc = psum(f"vp_{i}", "acc" if i % 2 == 0 else "avs", 1)
                for k in range(KCH):
                    lhs = xt[k][:, i * 128:(i + 1) * 128]
                    for half in range(2):
                        nc.tensor.matmul(
                            acc[:, half * 512:half * 512 + 384],
                            lhs,
                            wqkv_sb[k][:, 2 * INNER + half * 384:
                                       2 * INNER + (half + 1) * 384],
                            start=(k == 0), stop=(k == KCH - 1))
                for half in range(2):
                    dst = vaug[i].rearrange("p (h c) -> p h c", c=128)[
                        :, 6 * half:6 * (half + 1), 0:64]
                    nc.vector.tensor_copy(
                        dst, acc[:, half * 512:half * 512 + 384]
                        .rearrange("p (h d) -> p h d", d=64))

            # ---- interleaved: per pair t produce QK chunks (c=t, 6+t) with
            # rotary, then attention for pair t; the scheduler overlaps pair
            # t's attention with pair t+1's QK production.
            ao = [big.tile([128, N], F16, name=f"ao{t}", tag=f"ao{t}")
                  for t in range(TCH)]
            qkl = {}

            def emit_qk_pair(tq):
                for c in (tq, 6 + tq):
                    qkc = big.tile([128, N], F16, name=f"qk{c}",
                                   tag="qkA" if c < 6 else "qkB", bufs=2)
                    qkl[(tq, c >= 6)] = qkc
                    acc = psum(f"qkp_{c}", "acc", 1)
                    for k in range(KCH):
                        lhs = wqkv_sb[k][:, c * 128:(c + 1) * 128]
                        for half in range(2):
                            sl = slice(half * 512, (half + 1) * 512)
                            nc.tensor.matmul(acc[:, sl], lhs, xt[k][:, sl],
                                             start=(k == 0), stop=(k == KCH - 1))
                    qraw = tp.tile([128, N], F16, name=f"qraw_{c}",
                                   tag="qraw", bufs=2)
                    nc.vector.tensor_copy(qraw[:], acc[:])
                    # rotary: qkc <- qraw*cos + swap(qraw)*sin_signed, where
                    # the pair swap is two partition-strided SBUF DMAs and
                    # the per-row sign of rotate_every_two is folded into
                    # the host-built sin table.
                    rps = tp.tile([128, N], F16, name=f"rps_{c}", tag="rps",
                                  bufs=2)
                    qv = qraw.rearrange("(p s) n -> p s n", s=2)
                    rv = rps.rearrange("(p s) n -> p s n", s=2)
                    nc.scalar.dma_start(rv[:, 0, :], qv[:, 1, :])
                    nc.scalar.dma_start(rv[:, 1, :], qv[:, 0, :])
                    t1 = tp.tile([128, N], F16, name=f"t1_{c}", tag="t1",
                                 bufs=2)
                    nc.vector.tensor_mul(t1[:], qraw[:], cos_sb[:])
                    nc.vector.tensor_mul(qkc[:], rps[:], sin_sb[:])
                    nc.vector.tensor_add(qkc[:], qkc[:], t1[:])

            def emit_attention(t):
                # attention for pair t, one head at a time (one behind QK
                # production). avs is freed early via the av_sb copy so the
                # next head's accumulation overlaps this head's normalize.
                qkQ = qkl[(t, False)]
                qkK = qkl[(t, True)]
                for half in range(2):
                    h = 2 * t + half
                    hs = slice(half * 64, (half + 1) * 64)
                    avs = psum(f"av_{h}", "avs", 1)
                    for jc in range(NCH):
                        kt_slice = qkK[hs, jc * 128:(jc + 1) * 128]
                        e = epool.tile([128, N], BF16, name=f"e_{h}_{jc}",
                                       tag="e", bufs=4)
                        stp = psum(f"st_{h}_{jc}", "stp", 2)
                        for hf in range(2):
                            sl = slice(hf * 512, (hf + 1) * 512)
                            nc.tensor.matmul(stp[:, sl], kt_slice,
                                             qkQ[hs, sl],
                                             start=True, stop=True)
                        nc.scalar.activation(e[:], stp[:], AF.Exp, scale=SCALE)
                        v_sl = vaug[jc][:, h * 128:(h + 1) * 128]
                        for hf in range(2):
                            sl = slice(hf * 512, (hf + 1) * 512)
                            nc.tensor.matmul(avs[:, sl], v_sl, e[:, sl],
                                             start=(jc == 0),
                                             stop=(jc == NCH - 1))
                    # early evacuation: rows 0:64 = out^T, row 64 = denom
                    av_sb = tp.tile([65, N], F32, name=f"avsb_{h}",
                                    tag="avsb", bufs=2)
                    nc.vector.tensor_copy(av_sb[:], avs[0:65, :])
                    # normalize: den -> dram -> [128,8] recip -> dram -> row
                    # -> gpsimd bcast [64,N]; multiply into ao (fp16).
                    nc.sync.dma_start(den_d.ap()[h].unsqueeze(0),
                                      av_sb[64:65, :])
                    dsq = tp.tile([128, 8], F32, name=f"dsq_{h}", tag="dsq",
                                  bufs=2)
                    nc.sync.dma_start(
                        dsq[:], den_d.ap()[h].rearrange("(p f) -> p f", f=8))
                    nc.vector.reciprocal(dsq[:], dsq[:])
                    nc.sync.dma_start(
                        rcp_d.ap()[h].rearrange("(p f) -> p f", f=8), dsq[:])
                    rw = tp.tile([1, N], F32, name=f"rw_{h}", tag="rcp",
                                 bufs=2)
                    nc.sync.dma_start(rw[:], rcp_d.ap()[h].unsqueeze(0))
                    rep = tp.tile([64, N], F32, name=f"rep_{h}", tag="rep",
                                  bufs=2)
                    nc.gpsimd.partition_broadcast(rep[:], rw[:], channels=64)
                    nc.vector.tensor_mul(ao[t][hs, :], av_sb[0:64, :], rep[:])

            emit_qk_pair(0)
            for t in range(1, TCH):
                emit_qk_pair(t)
                emit_attention(t - 1)

            # ---- output projection pass A: heads 0..9 (k=0..4), emitted
            # before the last pair's attention so it fills that phase's PE
            # slack; partial sums land in SBUF fp32 accumulators.
            y_acc = [big.tile([128, DIM], F32, name=f"yacc{i}", tag=f"yacc{i}")
                     for i in range(NCH)]
            for i in range(NCH):
                op = psum(f"opA_{i}", "acc", 1)
                for k in range(TCH - 1):
                    lhs = ao[k][:, i * 128:(i + 1) * 128]
                    nc.tensor.matmul(op[:, 0:512], lhs, wout_sb[k][:, 0:512],
                                     start=(k == 0), stop=(k == TCH - 2))
                    nc.tensor.matmul(op[:, 512:768], lhs, wout_sb[k][:, 512:768],
                                     start=(k == 0), stop=(k == TCH - 2))
                nc.vector.tensor_add(y_acc[i][:], op[:, 0:768], b_bcast[:])

            emit_attention(TCH - 1)

            # ---- pass B: the last pair's contribution + store (short tail);
            # ping-pong acc/stp (attention is done, stp is free).
            for i in range(NCH):
                op = psum(f"opB_{i}", "acc" if i % 2 == 0 else "stp",
                          1 if i % 2 == 0 else 2)
                lhs = ao[TCH - 1][:, i * 128:(i + 1) * 128]
                nc.tensor.matmul(op[:, 0:512], lhs,
                                 wout_sb[TCH - 1][:, 0:512],
                                 start=True, stop=True)
                nc.tensor.matmul(op[:, 512:768], lhs,
                                 wout_sb[TCH - 1][:, 512:768],
                                 start=True, stop=True)
                nc.vector.tensor_add(y_acc[i][:], y_acc[i][:], op[:, 0:768])
                nc.sync.dma_start(y_d[i * 128:(i + 1) * 128, :], y_acc[i][:])

    nc.compile()
    return nc


def get_nc():
    if 'nc' not in _CACHE:
        _CACHE['nc'] = _build()
    return _CACHE['nc']


def make_in_maps(inputs):
    F16 = np.float16
    x = np.asarray(inputs["x"], dtype=np.float32)
    pos = np.asarray(inputs["pos_emb"], dtype=np.float32).reshape(N, DHEAD)
    wqkv = np.ascontiguousarray(
        np.asarray(inputs["W_qkv"], dtype=np.float32).astype(F16))
    wout = np.ascontiguousarray(
        np.asarray(inputs["W_out"], dtype=np.float32).astype(F16))
    bout = np.ascontiguousarray(np.asarray(inputs["b_out"], dtype=np.float32))
    # rotary tables in the transposed [d=128, n] layout used on-device:
    # row m of a head-half uses sin(pos[n, (m%64)//2]), cos(pos[n, 32+(m%64)//2]).
    # rotate_every_two's sign pattern (-odd, +even source) is folded into the
    # sin table since the device does an unsigned pair-swap copy.
    d = np.arange(128) % 64
    sgn = np.where(np.arange(128) % 2 == 0, -1.0, 1.0).astype(np.float32)
    sintab = np.ascontiguousarray((sgn[:, None] * pos[:, d // 2].T).astype(F16))
    costab = np.ascontiguousarray(pos[:, 32 + d // 2].T.astype(F16))
    return [{"xt": np.ascontiguousarray(x[i].T.astype(F16)),
             "wqkv": wqkv, "wout": wout, "bout": bout,
             "sintab": sintab, "costab": costab} for i in range(B)]


def run(inputs, trace=False, **kwargs):
    """inputs: dict with full-shape arrays as in reference.setup_inputs()."""
    from concourse.bass_utils import run_bass_kernel_spmd
    nc = get_nc()
    res = run_bass_kernel_spmd(nc, make_in_maps(inputs),
                               core_ids=list(range(B)), trace=trace, **kwargs)
    out = np.stack([res.results[i]["y"] for i in range(B)], axis=0)
    return out, res


def kernel(**inputs):
    out, _ = run(inputs, trace=False)
    return out


# revision 17
# speedup vs baseline: 1.7617x; 1.0479x over previous
"""Rotary multi-head attention (b=8, n=1024, dim=768, heads=12, d_head=64)
on 8 Trainium2 NeuronCores, data-parallel over batch (1 batch row per core).

v3: fp16 operands on the scores path (10-bit mantissa ~ tf32 accuracy, but
2-byte so every 128-col stationary gets the fast-weight-load path), bf16 for
exp outputs / V (exp can overflow fp16 range). Host-side prep: X shipped
pre-transposed, rotary sin/cos tables precomputed, weights pre-cast — the
device does no transposes and no weight staging. Attention runs per head
with a 3-tag PSUM budget (scores x2 | AV accumulator | matmul accumulator)
and the AV accumulator is freed early via a PSUM->SBUF copy so heads
pipeline; QK production for pair t+1 fills PE gaps under pair t's softmax.
"""
import sys
import numpy as np

if '/opt/trn_rl_repo' not in sys.path:
    sys.path.insert(0, '/opt/trn_rl_repo')

B, N, DIM = 8, 1024, 768
HEADS, DHEAD = 12, 64
INNER = HEADS * DHEAD           # 768
SCALE = DHEAD ** -0.5           # 0.125
NCH = N // 128                  # 8 n-chunks
KCH = DIM // 128                # 6 contraction chunks
TCH = HEADS // 2                # 6 head pairs

_CACHE = {}


def _build():
    import concourse.mybir as mybir
    from concourse import bacc
    from concourse.tile import TileContext

    F32 = mybir.dt.float32
    F16 = mybir.dt.float16
    BF16 = mybir.dt.bfloat16
    AF = mybir.ActivationFunctionType

    nc = bacc.Bacc("TRN2", target_bir_lowering=False, debug=False, num_devices=8)

    xt_d = nc.dram_tensor("xt", [DIM, N], F16, kind="ExternalInput")
    wqkv_d = nc.dram_tensor("wqkv", [DIM, 3 * INNER], F16, kind="ExternalInput")
    wout_d = nc.dram_tensor("wout", [INNER, DIM], F16, kind="ExternalInput")
    bout_d = nc.dram_tensor("bout", [DIM], F32, kind="ExternalInput")
    sin_d = nc.dram_tensor("sintab", [128, N], F16, kind="ExternalInput")
    cos_d = nc.dram_tensor("costab", [128, N], F16, kind="ExternalInput")
    y_d = nc.dram_tensor("y", [N, DIM], F32, kind="ExternalOutput")
    den_d = nc.dram_tensor("den_scr", [HEADS, N], F32)
    rcp_d = nc.dram_tensor("rcp_scr", [HEADS, N], F32)

    with TileContext(nc) as tc:
        with tc.tile_pool(name="wp", bufs=1) as wp, \
             tc.tile_pool(name="big", bufs=1) as big, \
             tc.tile_pool(name="tp", bufs=2) as tp, \
             tc.tile_pool(name="epool", bufs=1) as epool, \
             tc.tile_pool(name="misc", bufs=1) as misc, \
             tc.tile_pool(name="ps", bufs=1, space="PSUM") as ps:

            def psum(name, tag, bufs):
                return ps.tile([128, N], F32, name=name, tag=tag, bufs=bufs)

            # ---- input DMAs: interleave xt/wqkv pairs on opposite queues so
            # the k=0 chunks (needed first) land first on both queues.
            q_of = [nc.sync, nc.scalar]
            xt = [wp.tile([128, N], F16, name=f"xt{k}", tag=f"xt{k}")
                  for k in range(KCH)]
            wqkv_sb = [wp.tile([128, 3 * INNER], F16, name=f"wqkv_{k}",
                               tag=f"wqkv_{k}") for k in range(KCH)]
            for k in range(KCH):
                q_of[k % 2].dma_start(xt[k][:], xt_d[k * 128:(k + 1) * 128, :])
                q_of[(k + 1) % 2].dma_start(wqkv_sb[k][:],
                                            wqkv_d[k * 128:(k + 1) * 128, :])
            sin_sb = misc.tile([128, N], F16, name="sin_sb", tag="sin_sb")
            nc.sync.dma_start(sin_sb[:], sin_d.ap())
            cos_sb = misc.tile([128, N], F16, name="cos_sb", tag="cos_sb")
            nc.scalar.dma_start(cos_sb[:], cos_d.ap())

            wout_sb = [wp.tile([128, DIM], F16, name=f"wout_{k}",
                               tag=f"wout_{k}") for k in range(KCH)]
            for k in range(KCH):
                q_of[k % 2].dma_start(wout_sb[k][:],
                                      wout_d[k * 128:(k + 1) * 128, :])
            b_bcast = misc.tile([128, DIM], F32, name="b_bcast", tag="b_bcast")
            nc.scalar.dma_start(b_bcast[:],
                                bout_d.ap().unsqueeze(0).broadcast_to((128, DIM)))

            # ---- vaug: per n-chunk, [128, h*128 + (64 V | 1 ones | 63 junk)]
            # cols 65..127 are never read usefully (their AV output rows are
            # discarded), so only the ones column needs initialization.
            vaug = [big.tile([128, HEADS * 128], BF16, name=f"vaug{i}",
                             tag=f"vaug{i}") for i in range(NCH)]
            for i in range(NCH):
                ones_col = vaug[i].rearrange("p (h c) -> p h c", c=128)[
                    :, :, 64:65]
                nc.gpsimd.memset(ones_col, 1.0)

            # ---- V chunk emitters (natural layout, stationary = xt chunk).
            # Chunks are spread across PSUM tags and partly injected into the
            # first attention pair so the PE queue never head-of-line blocks.
            def emit_v_chunk(i, tag, bufs):
                acc = psum(f"vp_{i}", tag, bufs)
                for k in range(KCH):
                    lhs = xt[k][:, i * 128:(i + 1) * 128]
                    for half in range(2):
                        nc.tensor.matmul(
                            acc[:, half * 512:half * 512 + 384],
                            lhs,
                            wqkv_sb[k][:, 2 * INNER + half * 384:
                                       2 * INNER + (half + 1) * 384],
                            start=(k == 0), stop=(k == KCH - 1))
                for half in range(2):
                    dst = vaug[i].rearrange("p (h c) -> p h c", c=128)[
                        :, 6 * half:6 * (half + 1), 0:64]
                    nc.vector.tensor_copy(
                        dst, acc[:, half * 512:half * 512 + 384]
                        .rearrange("p (h d) -> p h d", d=64))

            def emit_v_pair_interleaved(i0, i1, tag):
                # two live PSUM tiles, k-loops interleaved so matmuls pace
                # with the per-k weight DMA arrivals instead of stalling on
                # the slowest chunk.
                a0 = psum(f"vp_{i0}", tag, 2)
                a1 = psum(f"vp_{i1}", tag, 2)
                for k in range(KCH):
                    for i, acc in ((i0, a0), (i1, a1)):
                        lhs = xt[k][:, i * 128:(i + 1) * 128]
                        for half in range(2):
                            nc.tensor.matmul(
                                acc[:, half * 512:half * 512 + 384],
                                lhs,
                                wqkv_sb[k][:, 2 * INNER + half * 384:
                                           2 * INNER + (half + 1) * 384],
                                start=(k == 0), stop=(k == KCH - 1))
                for i, acc in ((i0, a0), (i1, a1)):
                    for half in range(2):
                        dst = vaug[i].rearrange("p (h c) -> p h c", c=128)[
                            :, 6 * half:6 * (half + 1), 0:64]
                        nc.vector.tensor_copy(
                            dst, acc[:, half * 512:half * 512 + 384]
                            .rearrange("p (h d) -> p h d", d=64))

            # ---- interleaved: per pair t produce QK chunks (c=t, 6+t) with
            # rotary, then attention for pair t; the scheduler overlaps pair
            # t's attention with pair t+1's QK production.
            ao = [big.tile([128, N], F16, name=f"ao{t}", tag=f"ao{t}")
                  for t in range(TCH)]
            qkl = {}

            def emit_qk_pair(tq):
                for c in (tq, 6 + tq):
                    qkc = big.tile([128, N], F16, name=f"qk{c}",
                                   tag="qkA" if c < 6 else "qkB", bufs=2)
                    qkl[(tq, c >= 6)] = qkc
                    acc = psum(f"qkp_{c}", "acc", 1)
                    for k in range(KCH):
                        lhs = wqkv_sb[k][:, c * 128:(c + 1) * 128]
                        for half in range(2):
                            sl = slice(half * 512, (half + 1) * 512)
                            nc.tensor.matmul(acc[:, sl], lhs, xt[k][:, sl],
                                             start=(k == 0), stop=(k == KCH - 1))
                    qraw = tp.tile([128, N], F16, name=f"qraw_{c}",
                                   tag="qraw", bufs=2)
                    nc.vector.tensor_copy(qraw[:], acc[:])
                    # rotary: qkc <- qraw*cos + swap(qraw)*sin_signed, where
                    # the pair swap is two partition-strided SBUF DMAs and
                    # the per-row sign of rotate_every_two is folded into
                    # the host-built sin table.
                    rps = tp.tile([128, N], F16, name=f"rps_{c}", tag="rps",
                                  bufs=2)
                    qv = qraw.rearrange("(p s) n -> p s n", s=2)
                    rv = rps.rearrange("(p s) n -> p s n", s=2)
                    nc.scalar.dma_start(rv[:, 0, :], qv[:, 1, :])
                    nc.scalar.dma_start(rv[:, 1, :], qv[:, 0, :])
                    t1 = tp.tile([128, N], F16, name=f"t1_{c}", tag="t1",
                                 bufs=2)
                    nc.vector.tensor_mul(t1[:], qraw[:], cos_sb[:])
                    nc.vector.tensor_mul(qkc[:], rps[:], sin_sb[:])
                    nc.vector.tensor_add(qkc[:], qkc[:], t1[:])

            def emit_attention(t, filler=None):
                # attention for pair t, one head at a time (one behind QK
                # production). avs is freed early via the av_sb copy so the
                # next head's accumulation overlaps this head's normalize.
                qkQ = qkl[(t, False)]
                qkK = qkl[(t, True)]
                for half in range(2):
                    h = 2 * t + half
                    hs = slice(half * 64, (half + 1) * 64)
                    avs = psum(f"av_{h}", "avs", 1)
                    for jc in range(NCH):
                        if filler and (half, jc) in filler:
                            filler[(half, jc)]()
                        kt_slice = qkK[hs, jc * 128:(jc + 1) * 128]
                        e = epool.tile([128, N], BF16, name=f"e_{h}_{jc}",
                                       tag="e", bufs=4)
                        stp = psum(f"st_{h}_{jc}", "stp", 2)
                        for hf in range(2):
                            sl = slice(hf * 512, (hf + 1) * 512)
                            nc.tensor.matmul(stp[:, sl], kt_slice,
                                             qkQ[hs, sl],
                                             start=True, stop=True)
                        nc.scalar.activation(e[:], stp[:], AF.Exp, scale=SCALE)
                        v_sl = vaug[jc][:, h * 128:(h + 1) * 128]
                        for hf in range(2):
                            sl = slice(hf * 512, (hf + 1) * 512)
                            nc.tensor.matmul(avs[:, sl], v_sl, e[:, sl],
                                             start=(jc == 0),
                                             stop=(jc == NCH - 1))
                    # early evacuation: rows 0:64 = out^T, row 64 = denom
                    av_sb = tp.tile([65, N], F32, name=f"avsb_{h}",
                                    tag="avsb", bufs=2)
                    nc.vector.tensor_copy(av_sb[:], avs[0:65, :])
                    # normalize: den -> dram -> [128,8] recip -> dram -> row
                    # -> gpsimd bcast [64,N]; multiply into ao (fp16).
                    nc.sync.dma_start(den_d.ap()[h].unsqueeze(0),
                                      av_sb[64:65, :])
                    dsq = tp.tile([128, 8], F32, name=f"dsq_{h}", tag="dsq",
                                  bufs=2)
                    nc.sync.dma_start(
                        dsq[:], den_d.ap()[h].rearrange("(p f) -> p f", f=8))
                    nc.vector.reciprocal(dsq[:], dsq[:])
                    nc.sync.dma_start(
                        rcp_d.ap()[h].rearrange("(p f) -> p f", f=8), dsq[:])
                    rw = tp.tile([1, N], F32, name=f"rw_{h}", tag="rcp",
                                 bufs=2)
                    nc.sync.dma_start(rw[:], rcp_d.ap()[h].unsqueeze(0))
                    rep = tp.tile([64, N], F32, name=f"rep_{h}", tag="rep",
                                  bufs=2)
                    nc.gpsimd.partition_broadcast(rep[:], rw[:], channels=64)
                    nc.vector.tensor_mul(ao[t][hs, :], av_sb[0:64, :], rep[:])

            # Emission order tuned for pipeline fill: produce QK for pairs
            # 0+1 first (DMA-paced), V chunks 0-3 on the idle stp/avs slots,
            # then start attention 0 immediately with V chunks 4-7 injected
            # between its early iterations (they fill PE slack under exp).
            emit_qk_pair(0)
            emit_qk_pair(1)
            emit_v_pair_interleaved(0, 1, "stp")
            emit_v_chunk(2, "avs", 1)
            emit_v_chunk(3, "avs", 1)
            fillers = {(0, jc): (lambda i=4 + jc: emit_v_chunk(i, "acc", 1))
                       for jc in range(4)}
            emit_attention(0, filler=fillers)
            for t in range(2, TCH):
                emit_qk_pair(t)
                emit_attention(t - 1)

            # ---- output projection pass A: heads 0..9 (k=0..4), emitted
            # before the last pair's attention so it fills that phase's PE
            # slack; partial sums land in SBUF fp32 accumulators.
            y_acc = [big.tile([128, DIM], F32, name=f"yacc{i}", tag=f"yacc{i}")
                     for i in range(NCH)]
            for i in range(NCH):
                op = psum(f"opA_{i}", "acc", 1)
                for k in range(TCH - 1):
                    lhs = ao[k][:, i * 128:(i + 1) * 128]
                    nc.tensor.matmul(op[:, 0:512], lhs, wout_sb[k][:, 0:512],
                                     start=(k == 0), stop=(k == TCH - 2))
                    nc.tensor.matmul(op[:, 512:768], lhs, wout_sb[k][:, 512:768],
                                     start=(k == 0), stop=(k == TCH - 2))
                nc.vector.tensor_add(y_acc[i][:], op[:, 0:768], b_bcast[:])

            emit_attention(TCH - 1)

            # ---- pass B: the last pair's contribution + store (short tail);
            # ping-pong acc/stp (attention is done, stp is free).
            for i in range(NCH):
                op = psum(f"opB_{i}", "acc" if i % 2 == 0 else "stp",
                          1 if i % 2 == 0 else 2)
                lhs = ao[TCH - 1][:, i * 128:(i + 1) * 128]
                nc.tensor.matmul(op[:, 0:512], lhs,
                                 wout_sb[TCH - 1][:, 0:512],
                                 start=True, stop=True)
                nc.tensor.matmul(op[:, 512:768], lhs,
                                 wout_sb[TCH - 1][:, 512:768],
                                 start=True, stop=True)
                nc.vector.tensor_add(y_acc[i][:], y_acc[i][:], op[:, 0:768])
                nc.scalar.dma_start(y_d[i * 128:(i + 1) * 128, :], y_acc[i][:])

    nc.compile()
    return nc


def get_nc():
    if 'nc' not in _CACHE:
        _CACHE['nc'] = _build()
    return _CACHE['nc']


def make_in_maps(inputs):
    F16 = np.float16
    x = np.asarray(inputs["x"], dtype=np.float32)
    pos = np.asarray(inputs["pos_emb"], dtype=np.float32).reshape(N, DHEAD)
    wqkv = np.ascontiguousarray(
        np.asarray(inputs["W_qkv"], dtype=np.float32).astype(F16))
    wout = np.ascontiguousarray(
        np.asarray(inputs["W_out"], dtype=np.float32).astype(F16))
    bout = np.ascontiguousarray(np.asarray(inputs["b_out"], dtype=np.float32))
    # rotary tables in the transposed [d=128, n] layout used on-device:
    # row m of a head-half uses sin(pos[n, (m%64)//2]), cos(pos[n, 32+(m%64)//2]).
    # rotate_every_two's sign pattern (-odd, +even source) is folded into the
    # sin table since the device does an unsigned pair-swap copy.
    d = np.arange(128) % 64
    sgn = np.where(np.arange(128) % 2 == 0, -1.0, 1.0).astype(np.float32)
    sintab = np.ascontiguousarray((sgn[:, None] * pos[:, d // 2].T).astype(F16))
    costab = np.ascontiguousarray(pos[:, 32 + d // 2].T.astype(F16))
    return [{"xt": np.ascontiguousarray(x[i].T.astype(F16)),
             "wqkv": wqkv, "wout": wout, "bout": bout,
             "sintab": sintab, "costab": costab} for i in range(B)]


def run(inputs, trace=False, **kwargs):
    """inputs: dict with full-shape arrays as in reference.setup_inputs()."""
    from concourse.bass_utils import run_bass_kernel_spmd
    nc = get_nc()
    res = run_bass_kernel_spmd(nc, make_in_maps(inputs),
                               core_ids=list(range(B)), trace=trace, **kwargs)
    out = np.stack([res.results[i]["y"] for i in range(B)], axis=0)
    return out, res


def kernel(**inputs):
    out, _ = run(inputs, trace=False)
    return out
